# revision 11
# baseline (speedup 1.0000x reference)
"""Izhikevich 2-layer SNN on 8 Trainium2 cores — fp8 DoubleRow + fused-DVE design.

Reference (per timestep t of 100):
    cur1 = x_t @ W1.T + b1 ; spk1,v1,u1 = izh(cur1,v1,u1)
    cur2 = spk1 @ W2.T + b2 ; spk2,v2,u2 = izh(cur2,v2,u2)
    record spk2, v2   -> outputs [100, B, 10] each.

Data parallel over batch (2048 -> 8 x 256), weights replicated.

Device algorithm ("z-form", validated in sim_zform.py):
  states per layer row-space (layers stacked on partitions: 0:100 L1, 100:110 L2):
    z  = v + 75                  (stage_v columns; spike sentinel = C2 = 75.03)
    mu = u/(a*b) + 3750          (SBUF f32r tile, row 110 == 1.0 carries biases)
  per step:
    P    = W1@x (fp8 DoubleRow) + MU@mu (diag -ab + bias row) + SPKW@spk'  (psum)
    z'   = IZH_V(P, z_prev)      = select(W < C2, W, C2),
                                   W = P + 0.04*z_prev^2 - C1V*(z_prev >= C2)
    spk' = Sign(z' - DSIGN)      (ACT; +-1 encoding, halved W2 + bias fold)
    mu   = IZH_U(mu, z_prev)     = 0.98*mu + z_prev + K1*(z_prev >= C2)
  layer 2 is skewed 2 iterations behind layer 1 (102 iterations total), so the
  ACT/PE spike path never sits on the DVE critical cycle.
"""

import os
from contextlib import ExitStack

import numpy as np
import ml_dtypes

import concourse.bass as bass
import concourse.bacc as bacc
import concourse.mybir as mybir
import concourse.tile as tile
from concourse.bass_utils import run_bass_kernel_spmd

# ---------------- custom fused DVE ops ----------------
import concourse.dve_ops as dve_ops
from concourse.dve_spec import Spec, Src0, Src1, C0, C1, C2 as C2L, select, sq, lower, _has_src1
from concourse.dve_uop import DveOpSpec


def _mk_op(name, spec):
    for o in dve_ops.OPS:
        if o.name == name:
            return o
    row = dve_ops._CUSTOM_DVE_ROW_BASE + len(dve_ops.OPS)
    assert row < 0x20
    dve_ops._SUB_OPCODE_FOR_NAME[name] = row
    shas = {}
    for ver in ("v3", "v4"):
        d = DveOpSpec(name=name, opcode=row, uops=lower(spec, ver=ver),
                      rd1_en=_has_src1(spec))
        shas[ver] = d.sha(ver)
    op = dve_ops.DveOp(name, spec, subdim=False, uops_sha=shas)
    dve_ops.OPS.append(op)
    dve_ops.CUSTOM_DVE_SPECS[name] = spec
    return op


_WV = Src0 + C0 * sq(Src1) - C1 * (Src1 >= C2L)
IZH_V = _mk_op("IZH_V_ANT", Spec(
    body=select(_WV < C2L, _WV, C2L),
    reference=lambda in0, in1, s0, s1, imm2: np.where(
        (w := in0 + s0 * np.square(in1) - s1 * (in1 >= imm2)) < imm2, w, imm2
    ).astype(np.float32),
))
IZH_U = _mk_op("IZH_U_ANT", Spec(
    body=Src0 * C0 + Src1 + C1 * (Src1 >= C2L),
    reference=lambda in0, in1, s0, s1, imm2: (
        in0 * s0 + in1 + s1 * (in1 >= imm2)).astype(np.float32),
))

# ---------------- problem constants ----------------
A_, B_, Cr_, D_ = 0.02, 0.2, -65.0, 8.0
T, F, H, O = 100, 784, 100, 10
HO = H + O
NCORES = 8
BATCH = 2048
Bc = BATCH // NCORES        # 256
NIT = T + 2                 # layer-2 skew of 2
TB = 5                      # timesteps per x DMA slab (983 KB)
CH = 17                     # stage columns per buffer (6 * 17 = 102)
FMAIN = 768                 # features in the 3 main DoubleRow chunks
M_ = 112                    # padded out-columns for DR weight APs (step%16==0)

C2 = np.float32(75.03)
C0V = np.float32(0.04)
C1V = np.float32(0.04 * (np.float64(C2) ** 2 - 100.0) + 8.0)
C0U = np.float32(0.98)
K1 = np.float32(1960.0 + (10.0 - np.float64(C2)))
DSIGN = float(np.nextafter(C2, np.float32(0.0), dtype=np.float32))

F8 = ml_dtypes.float8_e4m3

LAST_RUN = None


def build_program(nc, ctx, tc):
    f32 = mybir.dt.float32
    f32r = mybir.dt.float32r
    f8 = mybir.dt.float8e4
    AF = mybir.ActivationFunctionType
    DR = mybir.MatmulPerfMode.DoubleRow

    xmain = nc.dram_tensor("xmain", [T // TB, 128, TB * 1536], f8,
                           kind="ExternalInput").ap()
    xrunt = nc.dram_tensor("xrunt", [T // TB, 8, TB * 512], f8,
                           kind="ExternalInput").ap()
    wmain = nc.dram_tensor("wmain", [128, 3 * 2 * M_], f8, kind="ExternalInput").ap()
    wrunt = nc.dram_tensor("wrunt", [8, 2 * M_], f8, kind="ExternalInput").ap()
    wmu = nc.dram_tensor("wmu", [HO + 1, HO], f32r, kind="ExternalInput").ap()
    wspk = nc.dram_tensor("wspk", [H, HO], f32r, kind="ExternalInput").ap()
    muini = nc.dram_tensor("muini", [HO + 1, Bc], f32r, kind="ExternalInput").ap()
    out = nc.dram_tensor("out", [2, O, T, Bc], f32, kind="ExternalOutput").ap()

    const = ctx.enter_context(tc.tile_pool(name="const", bufs=1))
    state = ctx.enter_context(tc.tile_pool(name="state", bufs=1))
    xpool = ctx.enter_context(tc.tile_pool(name="x", bufs=3))
    rpool = ctx.enter_context(tc.tile_pool(name="xr", bufs=3))
    svpool = ctx.enter_context(tc.tile_pool(name="sv", bufs=2))
    sspool = ctx.enter_context(tc.tile_pool(name="ss", bufs=2))
    pp = ctx.enter_context(tc.tile_pool(name="ps", bufs=6, space="PSUM"))

    wmain_sb = const.tile([128, 3 * 2 * M_], f8)
    nc.sync.dma_start(wmain_sb[:], wmain)
    wrunt_sb = const.tile([8, 2 * M_], f8)
    nc.sync.dma_start(wrunt_sb[:], wrunt)
    wmu_sb = const.tile([HO + 1, HO], f32r)
    nc.sync.dma_start(wmu_sb[:], wmu)
    wspk_sb = const.tile([H, HO], f32r)
    nc.sync.dma_start(wspk_sb[:], wspk)
    zinit = const.tile([HO, Bc], f32)
    nc.vector.memset(zinit[:], 5.0)
    dsgn = const.tile([HO, 1], f32)
    nc.vector.memset(dsgn[:], -DSIGN)

    mu = state.tile([HO + 1, Bc], f32r)
    nc.sync.dma_start(mu[:], muini)

    wchunks = [
        wmain_sb[:, c * 2 * M_:(c + 1) * 2 * M_].rearrange("p (two m) -> p two m", two=2)
        for c in range(3)
    ]
    wruntap = wrunt_sb[:].rearrange("p (two m) -> p two m", two=2)

    sv_tiles = {}   # buffer index -> tile
    ss_tiles = {}
    xt = xr = None

    def vcol(i):
        b, c = divmod(i, CH)
        return sv_tiles[b][:, c * Bc:(c + 1) * Bc]

    def scol(i):
        b, c = divmod(i, CH)
        return ss_tiles[b][:, c * Bc:(c + 1) * Bc]

    for i in range(NIT):
        blk, s = divmod(i, TB)
        if i < T and s == 0:
            xt = xpool.tile([128, TB * 1536], f8, tag="xt")
            nc.sync.dma_start(xt[:], xmain[blk])
            xr = rpool.tile([8, TB * 512], f8, tag="xr")
            nc.sync.dma_start(xr[:], xrunt[blk])
        b, c = divmod(i, CH)
        if c == 0:
            sv_tiles[b] = svpool.tile([HO, CH * Bc], f32, tag="sv", name="svt")
            ss_tiles[b] = sspool.tile([HO, CH * Bc], f32r, tag="ss", name="sst")
            if i == 0:
                # cols 0,1 read as initial z (=5.0) where IZH_V doesn't overwrite
                nc.vector.memset(sv_tiles[0][:, 0:2 * Bc], 5.0)

        ps = pp.tile([M_, Bc], f32)
        if i < T:
            for cc in range(3):
                nc.tensor.matmul(
                    ps[:],
                    wchunks[cc],
                    xt[:, (s * 3 + cc) * 512:(s * 3 + cc + 1) * 512]
                    .rearrange("p (two n) -> p two n", two=2),
                    start=(cc == 0), stop=False, perf_mode=DR)
            nc.tensor.matmul(
                ps[:], wruntap,
                xr[:, s * 512:(s + 1) * 512].rearrange("p (two n) -> p two n", two=2),
                start=False, stop=False, perf_mode=DR)
            nc.tensor.matmul(ps[0:HO, :], wmu_sb[:], mu[:],
                             start=False, stop=(i < 2))
        else:
            nc.tensor.matmul(ps[0:HO, :], wmu_sb[:], mu[:], start=True, stop=False)
        if i >= 2:
            nc.tensor.matmul(ps[0:HO, :], wspk_sb[:], scol(i - 2)[0:H, :],
                             start=False, stop=True)

        hi = H if i < 2 else HO
        vp = zinit[:] if i == 0 else vcol(i - 1)
        vo = vcol(i)
        nc.vector._custom_dve(IZH_V, out=vo[0:hi, :], in0=ps[0:hi, :],
                              in1=vp[0:hi, :], s0=float(C0V), s1=float(C1V),
                              imm2=float(C2))
        nc.scalar.activation(scol(i)[:], vo, AF.Sign, bias=dsgn[:, 0:1], scale=1.0)
        if i < 2:
            nc.vector._custom_dve(IZH_U, out=mu[0:H, :], in0=mu[0:H, :],
                                  in1=vp[0:H, :], s0=float(C0U), s1=float(K1),
                                  imm2=float(C2))
        elif i < NIT - 1:
            nc.vector._custom_dve(IZH_U, out=mu[0:HO, :], in0=mu[0:HO, :],
                                  in1=vp[0:HO, :], s0=float(C0U), s1=float(K1),
                                  imm2=float(C2))

        if c == CH - 1 or i == NIT - 1:
            j0 = b * CH
            cs = 2 - j0 if j0 < 2 else 0      # skip cols 0,1 (inits)
            ncols = c + 1 - cs
            t0 = j0 + cs - 2
            nc.sync.dma_start(
                out[0, :, t0:t0 + ncols, :],
                ss_tiles[b][H:HO, cs * Bc:(c + 1) * Bc]
                .bitcast(f32).rearrange("p (t b) -> p t b", t=ncols))
            nc.sync.dma_start(
                out[1, :, t0:t0 + ncols, :],
                sv_tiles[b][H:HO, cs * Bc:(c + 1) * Bc]
                .rearrange("p (t b) -> p t b", t=ncols))


def _host_inputs(x, W1, b1, W2, b2):
    """Quantize + pack per-core inputs."""
    xq = np.ascontiguousarray(x, np.float32).astype(F8)      # [2048, 100, 784]
    W1q = np.asarray(W1, np.float32).astype(F8)              # [100, 784]
    W2f = np.asarray(W2, np.float64)
    b1f = np.asarray(b1, np.float64)
    b2f = np.asarray(b2, np.float64)

    # weights: chunk c, pair-row k, pair p -> feature f = c*256 + k*2 + p
    wm = np.zeros((128, 3, 2, M_), F8)
    wmf = W1q[:, :FMAIN].reshape(H, 3, 128, 2)               # [m, c, k, p]
    wm[:, :, :, :H] = wmf.transpose(2, 1, 3, 0)
    wr = np.zeros((8, 2, M_), F8)
    wrf = W1q[:, FMAIN:].reshape(H, 8, 2)
    wr[:, :, :H] = wrf.transpose(1, 2, 0)

    gamma = np.zeros(HO, np.float64)
    gamma[:H] = b1f + 5.0
    gamma[H:] = b2f + 5.0 + 0.5 * W2f.sum(axis=1)
    wmu = np.zeros((HO + 1, HO), np.float32)
    wmu[np.arange(HO), np.arange(HO)] = -A_ * B_
    wmu[HO, :] = gamma
    wspk = np.zeros((H, HO), np.float32)
    wspk[:, H:] = 0.5 * W2f.T
    muini_h = np.zeros((HO + 1, Bc), np.float32)
    muini_h[HO, :] = 1.0

    in_maps = []
    for i in range(NCORES):
        xs = xq[i * Bc:(i + 1) * Bc]                         # [256, 100, 784] f8
        # main: [T//TB, 128, TB, 3, 2, 256] -> [T//TB, 128, TB*1536]
        xmf = xs[:, :, :FMAIN].reshape(Bc, T // TB, TB, 3, 128, 2)
        xmain = np.ascontiguousarray(
            xmf.transpose(1, 4, 2, 3, 5, 0)).reshape(T // TB, 128, TB * 1536)
        xrf = xs[:, :, FMAIN:].reshape(Bc, T // TB, TB, 8, 2)
        xrunt = np.ascontiguousarray(
            xrf.transpose(1, 3, 2, 4, 0)).reshape(T // TB, 8, TB * 512)
        in_maps.append({
            "xmain": xmain, "xrunt": xrunt,
            "wmain": wm.reshape(128, 3 * 2 * M_), "wrunt": wr.reshape(8, 2 * M_),
            "wmu": wmu, "wspk": wspk, "muini": muini_h,
        })
    return in_maps


def _install_ntff_shim():
    import sys
    import types
    try:
        import antenv.axon_hooks  # noqa: F401
        return
    except ImportError:
        pass
    try:
        from trn_agent_boot.trn_boot import _ntff_profile_via_ctypes
        hook = _ntff_profile_via_ctypes("/opt/axon/libaxon_pjrt.so")
        mod = types.ModuleType("antenv.axon_hooks")
        mod._hook = hook
        mod.get_axon_ntff_profile_hook = lambda: mod._hook
        mod.set_axon_ntff_profile_hook = lambda h: setattr(mod, "_hook", h)
        sys.modules["antenv.axon_hooks"] = mod
    except Exception:
        pass


def kernel(x, W1, b1, W2, b2):
    global LAST_RUN
    if os.environ.get("BASS_TRACE"):
        _install_ntff_shim()

    nc = bacc.Bacc("TRN2", target_bir_lowering=False, debug=False,
                   num_devices=NCORES)
    with tile.TileContext(nc) as tc:
        with ExitStack() as ctx:
            build_program(nc, ctx, tc)
    nc.compile()

    in_maps = _host_inputs(x, W1, b1, W2, b2)
    res = run_bass_kernel_spmd(
        nc, in_maps, core_ids=list(range(NCORES)),
        trace=bool(os.environ.get("BASS_TRACE")),
    )
    LAST_RUN = res

    spk = np.empty((T, BATCH, O), np.float32)
    mem = np.empty((T, BATCH, O), np.float32)
    for i in range(NCORES):
        o = res.results[i]["out"]                # [2, O, T, Bc]
        sp = (o[0] > 0.0).astype(np.float32)     # sign -> {0,1}
        zz = o[1]
        mm = np.where(sp > 0, np.float32(Cr_), zz - np.float32(75.0))
        spk[:, i * Bc:(i + 1) * Bc, :] = sp.transpose(1, 2, 0)
        mem[:, i * Bc:(i + 1) * Bc, :] = mm.transpose(1, 2, 0)
    return spk, mem


# revision 14
# speedup vs baseline: 1.1451x; 1.1451x over previous
"""Izhikevich 2-layer SNN on 8 Trainium2 cores — fp8 DoubleRow + fused-DVE design.

Reference (per timestep t of 100):
    cur1 = x_t @ W1.T + b1 ; spk1,v1,u1 = izh(cur1,v1,u1)
    cur2 = spk1 @ W2.T + b2 ; spk2,v2,u2 = izh(cur2,v2,u2)
    record spk2, v2   -> outputs [100, B, 10] each.

Data parallel over batch (2048 -> 8 x 256), weights replicated.

Device algorithm ("z-form", validated in sim_zform.py):
  states per layer row-space (layers stacked on partitions: 0:100 L1, 100:110 L2):
    z  = v + 75                  (stage_v columns; spike sentinel = C2 = 75.03)
    mu = u/(a*b) + 3750          (SBUF f32r tile, row 110 == 1.0 carries biases)
  per step:
    P    = W1@x (fp8 DoubleRow) + MU@mu (diag -ab + bias row) + SPKW@spk'  (psum)
    z'   = IZH_V(P, z_prev)      = select(W < C2, W, C2),
                                   W = P + 0.04*z_prev^2 - C1V*(z_prev >= C2)
    spk' = Sign(z' - DSIGN)      (ACT; +-1 encoding, halved W2 + bias fold)
    mu   = IZH_U(mu, z_prev)     = 0.98*mu + z_prev + K1*(z_prev >= C2)
  layer 2 is skewed 2 iterations behind layer 1 (102 iterations total), so the
  ACT/PE spike path never sits on the DVE critical cycle.
"""

import os
from contextlib import ExitStack

import numpy as np
import ml_dtypes

import concourse.bass as bass
import concourse.bacc as bacc
import concourse.mybir as mybir
import concourse.tile as tile
from concourse.bass_utils import run_bass_kernel_spmd

# ---------------- custom fused DVE ops ----------------
import concourse.dve_ops as dve_ops
from concourse.dve_spec import Spec, Src0, Src1, C0, C1, C2 as C2L, select, sq, lower, _has_src1
from concourse.dve_uop import DveOpSpec


def _mk_op(name, spec):
    for o in dve_ops.OPS:
        if o.name == name:
            return o
    row = dve_ops._CUSTOM_DVE_ROW_BASE + len(dve_ops.OPS)
    assert row < 0x20
    dve_ops._SUB_OPCODE_FOR_NAME[name] = row
    shas = {}
    for ver in ("v3", "v4"):
        d = DveOpSpec(name=name, opcode=row, uops=lower(spec, ver=ver),
                      rd1_en=_has_src1(spec))
        shas[ver] = d.sha(ver)
    op = dve_ops.DveOp(name, spec, subdim=False, uops_sha=shas)
    dve_ops.OPS.append(op)
    dve_ops.CUSTOM_DVE_SPECS[name] = spec
    return op


_WV = Src0 + C0 * sq(Src1) - C1 * (Src1 >= C2L)
IZH_V = _mk_op("IZH_V_ANT", Spec(
    body=select(_WV < C2L, _WV, C2L),
    reference=lambda in0, in1, s0, s1, imm2: np.where(
        (w := in0 + s0 * np.square(in1) - s1 * (in1 >= imm2)) < imm2, w, imm2
    ).astype(np.float32),
))
IZH_U = _mk_op("IZH_U_ANT", Spec(
    body=Src0 * C0 + Src1 + C1 * (Src1 >= C2L),
    reference=lambda in0, in1, s0, s1, imm2: (
        in0 * s0 + in1 + s1 * (in1 >= imm2)).astype(np.float32),
))

# ---------------- problem constants ----------------
A_, B_, Cr_, D_ = 0.02, 0.2, -65.0, 8.0
T, F, H, O = 100, 784, 100, 10
HO = H + O
NCORES = 8
BATCH = 2048
Bc = BATCH // NCORES        # 256
NIT = T + 2                 # layer-2 skew of 2
TB = 5                      # timesteps per x DMA slab (983 KB)
CH = 17                     # stage columns per buffer (6 * 17 = 102)
FMAIN = 768                 # features in the 3 main DoubleRow chunks
M_ = 112                    # padded out-columns for DR weight APs (step%16==0)

C2 = np.float32(75.03)
C0V = np.float32(0.04)
C1V = np.float32(0.04 * (np.float64(C2) ** 2 - 100.0) + 8.0)
C0U = np.float32(0.98)
K1 = np.float32(1960.0 + (10.0 - np.float64(C2)))
DSIGN = float(np.nextafter(C2, np.float32(0.0), dtype=np.float32))

F8 = ml_dtypes.float8_e4m3

LAST_RUN = None


def build_program(nc, ctx, tc):
    f32 = mybir.dt.float32
    f32r = mybir.dt.float32r
    bf16 = mybir.dt.bfloat16
    f8 = mybir.dt.float8e4
    AF = mybir.ActivationFunctionType
    DR = mybir.MatmulPerfMode.DoubleRow

    xmain = nc.dram_tensor("xmain", [T // TB, 128, TB * 1536], f8,
                           kind="ExternalInput").ap()
    xrunt = nc.dram_tensor("xrunt", [8, 2 * 512], f8, kind="ExternalInput").ap()
    xrbig = nc.dram_tensor("xrbig", [16, NIT * Bc], bf16, kind="ExternalInput").ap()
    wmain = nc.dram_tensor("wmain", [128, 3 * 2 * M_], f8, kind="ExternalInput").ap()
    wrunt = nc.dram_tensor("wrunt", [8, 2 * M_], f8, kind="ExternalInput").ap()
    wmu = nc.dram_tensor("wmu", [HO + 1, HO], f32r, kind="ExternalInput").ap()
    wspk = nc.dram_tensor("wspk", [126, HO], bf16, kind="ExternalInput").ap()
    muini = nc.dram_tensor("muini", [HO + 1, Bc], f32r, kind="ExternalInput").ap()
    outs = nc.dram_tensor("outs", [O, T, Bc], bf16, kind="ExternalOutput").ap()
    outv = nc.dram_tensor("outv", [O, T, Bc], f32, kind="ExternalOutput").ap()

    const = ctx.enter_context(tc.tile_pool(name="const", bufs=1))
    state = ctx.enter_context(tc.tile_pool(name="state", bufs=1))
    xpool = ctx.enter_context(tc.tile_pool(name="x", bufs=3))
    svpool = ctx.enter_context(tc.tile_pool(name="sv", bufs=2))
    sspool = ctx.enter_context(tc.tile_pool(name="ss", bufs=2))
    pp = ctx.enter_context(tc.tile_pool(name="ps", bufs=8, space="PSUM"))

    wmain_sb = const.tile([128, 3 * 2 * M_], f8)
    nc.sync.dma_start(wmain_sb[:], wmain)
    wrunt_sb = const.tile([8, 2 * M_], f8)
    nc.sync.dma_start(wrunt_sb[:], wrunt)
    xrunt_sb = const.tile([8, 2 * 512], f8)
    nc.sync.dma_start(xrunt_sb[:], xrunt)
    wmu_sb = const.tile([HO + 1, HO], f32r)
    nc.sync.dma_start(wmu_sb[:], wmu)
    wspk_sb = const.tile([126, HO], bf16)
    nc.sync.dma_start(wspk_sb[:], wspk)
    zinit = const.tile([HO, Bc], f32)
    nc.vector.memset(zinit[:], 5.0)
    dsgn = const.tile([HO, 1], f32)
    nc.vector.memset(dsgn[:], -DSIGN)

    mu = state.tile([HO + 1, Bc], f32r)
    nc.sync.dma_start(mu[:], muini)

    wchunks = [
        wmain_sb[:, c * 2 * M_:(c + 1) * 2 * M_].rearrange("p (two m) -> p two m", two=2)
        for c in range(3)
    ]
    wruntap = wrunt_sb[:].rearrange("p (two m) -> p two m", two=2)

    sv_tiles = {}   # buffer index -> tile
    ss_tiles = {}
    xt = xr = None

    def vcol(i):
        b, c = divmod(i, CH)
        return sv_tiles[b][:, c * Bc:(c + 1) * Bc]

    def scol(i):
        b, c = divmod(i, CH)
        return ss_tiles[b][:, c * Bc:(c + 1) * Bc]

    ps_tiles = {}

    def emit_step(i):
        b, c = divmod(i, CH)
        if c == 0:
            sv_tiles[b] = svpool.tile([HO, CH * Bc], f32, tag="sv", name="svt")
            ss_tiles[b] = sspool.tile([126, CH * Bc], bf16, tag="ss", name="sst")
            nc.sync.dma_start(ss_tiles[b][110:126, :],
                              xrbig[:, b * CH * Bc:(b + 1) * CH * Bc])
            if i == 0:
                # cols 0,1 read as initial z (=5.0) where IZH_V doesn't overwrite
                nc.vector.memset(sv_tiles[0][:, 0:2 * Bc], 5.0)

        ps = ps_tiles.pop(i)
        if i < T:
            nc.tensor.matmul(ps[0:HO, :], wmu_sb[:], mu[:],
                             start=False, stop=False)
        else:
            nc.tensor.matmul(ps[0:HO, :], wmu_sb[:], mu[:], start=True,
                             stop=False)
        if i >= 2:
            nc.tensor.matmul(ps[0:HO, :], wspk_sb[:],
                             ss_tiles[(i - 2) // CH][0:126,
                                                     ((i - 2) % CH) * Bc:
                                                     ((i - 2) % CH + 1) * Bc],
                             start=False, stop=True)
        else:
            # steps 0,1: runt features via tiny fp8 DoubleRow matmul instead
            nc.tensor.matmul(
                ps[:], wruntap,
                xrunt_sb[:, i * 512:(i + 1) * 512]
                .rearrange("p (two n) -> p two n", two=2),
                start=False, stop=True, perf_mode=DR)

        hi = H if i < 2 else HO
        vp = zinit[:] if i == 0 else vcol(i - 1)
        vo = vcol(i)
        nc.vector._custom_dve(IZH_V, out=vo[0:hi, :], in0=ps[0:hi, :],
                              in1=vp[0:hi, :], s0=float(C0V), s1=float(C1V),
                              imm2=float(C2))
        nc.scalar.activation(scol(i)[0:HO, :], vo, AF.Sign, bias=dsgn[:, 0:1],
                             scale=1.0)
        if i < 2:
            nc.vector._custom_dve(IZH_U, out=mu[0:H, :], in0=mu[0:H, :],
                                  in1=vp[0:H, :], s0=float(C0U), s1=float(K1),
                                  imm2=float(C2))
        elif i < NIT - 1:
            nc.vector._custom_dve(IZH_U, out=mu[0:HO, :], in0=mu[0:HO, :],
                                  in1=vp[0:HO, :], s0=float(C0U), s1=float(K1),
                                  imm2=float(C2))

        if c == CH - 1 or i == NIT - 1:
            j0 = b * CH
            cs = 2 - j0 if j0 < 2 else 0      # skip cols 0,1 (inits)
            ncols = c + 1 - cs
            t0 = j0 + cs - 2
            nc.sync.dma_start(
                outs[:, t0:t0 + ncols, :],
                ss_tiles[b][H:HO, cs * Bc:(c + 1) * Bc]
                .rearrange("p (t b) -> p t b", t=ncols))
            nc.sync.dma_start(
                outv[:, t0:t0 + ncols, :],
                sv_tiles[b][H:HO, cs * Bc:(c + 1) * Bc]
                .rearrange("p (t b) -> p t b", t=ncols))

    for blk in range(T // TB):
        xt = xpool.tile([128, TB * 1536], f8, tag="xt")
        nc.sync.dma_start(xt[:], xmain[blk])
        for s_ in range(TB):
            ps_tiles[blk * TB + s_] = pp.tile([M_, Bc], f32, name="pst")
        for cc in range(3):
            for s_ in range(TB):
                nc.tensor.matmul(
                    ps_tiles[blk * TB + s_][:],
                    wchunks[cc],
                    xt[:, (s_ * 3 + cc) * 512:(s_ * 3 + cc + 1) * 512]
                    .rearrange("p (two n) -> p two n", two=2),
                    start=(cc == 0), stop=False, perf_mode=DR)
        for s_ in range(TB):
            emit_step(blk * TB + s_)
    for i in range(T, NIT):
        ps_tiles[i] = pp.tile([M_, Bc], f32, name="pst")
        emit_step(i)


def _host_inputs(x, W1, b1, W2, b2):
    """Quantize + pack per-core inputs."""
    BF = ml_dtypes.bfloat16
    xf = np.ascontiguousarray(x, np.float32)
    xq = xf.astype(F8)                                       # [2048, 100, 784]
    W1q = np.asarray(W1, np.float32).astype(F8)              # [100, 784]
    W2f = np.asarray(W2, np.float64)
    b1f = np.asarray(b1, np.float64)
    b2f = np.asarray(b2, np.float64)

    # main weights: chunk c, pair-row k, pair p -> feature f = c*256 + k*2 + p
    wm = np.zeros((128, 3, 2, M_), F8)
    wmf = W1q[:, :FMAIN].reshape(H, 3, 128, 2)               # [m, c, k, p]
    wm[:, :, :, :H] = wmf.transpose(2, 1, 3, 0)
    wr = np.zeros((8, 2, M_), F8)
    wrf = W1q[:, FMAIN:].reshape(H, 8, 2)
    wr[:, :, :H] = wrf.transpose(1, 2, 0)

    # spk/runt matmul lhsT [126, 110] bf16
    w2h = (0.5 * W2f).astype(BF)                             # [10, 100]
    wspk = np.zeros((126, HO), BF)
    wspk[:H, H:] = w2h.T
    wspk[110:126, :H] = np.asarray(W1, np.float32)[:, FMAIN:].T.astype(BF)

    gamma = np.zeros(HO, np.float64)
    gamma[:H] = b1f + 5.0
    gamma[H:] = b2f + 5.0 + w2h.astype(np.float64).sum(axis=1)
    wmu = np.zeros((HO + 1, HO), np.float32)
    wmu[np.arange(HO), np.arange(HO)] = -A_ * B_
    wmu[HO, :] = gamma
    muini_h = np.zeros((HO + 1, Bc), np.float32)
    muini_h[HO, :] = 1.0

    in_maps = []
    for i in range(NCORES):
        xs = xq[i * Bc:(i + 1) * Bc]                         # [256, 100, 784] f8
        xmf = xs[:, :, :FMAIN].reshape(Bc, T // TB, TB, 3, 128, 2)
        xmain = np.ascontiguousarray(
            xmf.transpose(1, 4, 2, 3, 5, 0)).reshape(T // TB, 128, TB * 1536)
        # runt steps 0,1 (fp8 DoubleRow): [k, s, p, n]
        xr2 = np.ascontiguousarray(
            xs[:, 0:2, FMAIN:].reshape(Bc, 2, 8, 2).transpose(2, 1, 3, 0)
        ).reshape(8, 2 * 512)
        # runt steps 2..99 as bf16 rows of the spike tile, shifted by the skew
        xrb = np.zeros((16, NIT, Bc), BF)
        xrb[:, 0:T - 2, :] = xf[i * Bc:(i + 1) * Bc, 2:T, FMAIN:]\
            .transpose(2, 1, 0).astype(BF)
        in_maps.append({
            "xmain": xmain, "xrunt": xr2, "xrbig": xrb.reshape(16, NIT * Bc),
            "wmain": wm.reshape(128, 3 * 2 * M_), "wrunt": wr.reshape(8, 2 * M_),
            "wmu": wmu, "wspk": wspk, "muini": muini_h,
        })
    return in_maps


def _install_ntff_shim():
    import sys
    import types
    try:
        import antenv.axon_hooks  # noqa: F401
        return
    except ImportError:
        pass
    try:
        from trn_agent_boot.trn_boot import _ntff_profile_via_ctypes
        hook = _ntff_profile_via_ctypes("/opt/axon/libaxon_pjrt.so")
        mod = types.ModuleType("antenv.axon_hooks")
        mod._hook = hook
        mod.get_axon_ntff_profile_hook = lambda: mod._hook
        mod.set_axon_ntff_profile_hook = lambda h: setattr(mod, "_hook", h)
        sys.modules["antenv.axon_hooks"] = mod
    except Exception:
        pass


def kernel(x, W1, b1, W2, b2):
    global LAST_RUN
    if os.environ.get("BASS_TRACE"):
        _install_ntff_shim()

    nc = bacc.Bacc("TRN2", target_bir_lowering=False, debug=False,
                   num_devices=NCORES)
    with tile.TileContext(nc) as tc:
        with ExitStack() as ctx:
            build_program(nc, ctx, tc)
    nc.compile()

    in_maps = _host_inputs(x, W1, b1, W2, b2)
    res = run_bass_kernel_spmd(
        nc, in_maps, core_ids=list(range(NCORES)),
        trace=bool(os.environ.get("BASS_TRACE")),
    )
    LAST_RUN = res

    spk = np.empty((T, BATCH, O), np.float32)
    mem = np.empty((T, BATCH, O), np.float32)
    for i in range(NCORES):
        sp = (res.results[i]["outs"].astype(np.float32) > 0.0).astype(np.float32)
        zz = res.results[i]["outv"]                          # [O, T, Bc]
        mm = np.where(sp > 0, np.float32(Cr_), zz - np.float32(75.0))
        spk[:, i * Bc:(i + 1) * Bc, :] = sp.transpose(1, 2, 0)
        mem[:, i * Bc:(i + 1) * Bc, :] = mm.transpose(1, 2, 0)
    return spk, mem
